# revision 29
# baseline (speedup 1.0000x reference)
"""Trainium2 Bass kernel for nn_BoundaryDiceLoss_82171314307268.

Sharding: pure data-parallel over 8 cores; core c handles sample c//2,
D-half c%2. Each core gets a [H=128(partitions), 70 D-slots, 128 w] slab
(64 owned D slices + 3 halo, out-of-volume D replicated with edge
values) of out0/out1 (fp16 logits) plus target fields.

Per-core algorithm (fused chunk pipeline):
  diff = out1 - out0 (fp16, DVE);  probs = sigmoid(diff) (ACT, bf16)
  v = (diff > 0) + 8*target in {0,1,8,9} fp8e4 (DVE stt; all e4m3-exact).
  Boundary: L = cross-sum(v) - 6v;  |L_p-part| <= 6 < 8 <= |8*L_t-part|
    when t-boundary, so L != 0 iff either mask is non-constant on its
    in-volume 6-neighborhood (edge-replicated padding: host D-halo,
    per-chunk w-pad copies, A1 edge rows).  The 5 cross-terms run as 3
    fp8e4 DoubleRow matmul pairs (the pair is an extra AP dim over the
    same v tile at two shift offsets; PE is output-column-bound at
    2.4 GHz so pairing halves instruction count).
    E = Square(L/8) -> fp8e5, > 0 exactly on the two-sided boundary.
  region r = conv3d(E, ball radius 2): 13 (dz,dw) shift-terms as 7
    fp8e5 DoubleRow matmuls accumulating one PSUM chunk; r > 0 is
    exactly the reference's dilation mask (all terms >= 0).
  Products: m = Sign(r) in {0,1} (ACT, exact since r >= 0), pm = m*probs,
    mt = m*t, ptm = pm*t (DVE bf16 TTs); reduced on PE by an all-ones
    bf16 matmul per field accumulating across all 16 owned chunks in
    PSUM (delayed one step so PE never waits on fresh DVE output);
    final 3 DVE accum_out -> [128,3] (all partitions identical).
  The reference's nonempty gate is algebraically redundant (empty
  region => sums exactly 0 => dice = eps/eps = 1 => loss term 0), so
  S_m is never computed.

Engine notes: fp8 DoubleRow = 0.5 PE cycles/row (two matmul terms per
instruction); DVE perf modes need all-2-byte packed SBUF operands
(fp8/f32/PSUM operands drop to 1x); PE runs 2.4 GHz only when kept
continuously busy, hence one fused software-pipelined loop rather than
phase barriers. Input DMAs + tiny pad copies issue from the idle Pool
queue (25ns dispatch vs 625ns on sync).
"""
import sys

sys.path.insert(0, "/opt/trn_rl_repo")

import numpy as np
import ml_dtypes

import concourse.bass as bass
import concourse.bacc as bacc
import concourse.tile as tile
import concourse.mybir as mybir
from concourse.bass_utils import run_bass_kernel_spmd

f32 = mybir.dt.float32
bf16 = mybir.dt.bfloat16
fp16 = mybir.dt.float16
fp8e4 = mybir.dt.float8e4
fp8e5 = mybir.dt.float8e5
Alu = mybir.AluOpType
Act = mybir.ActivationFunctionType
DR = mybir.MatmulPerfMode.DoubleRow

P = 128          # H on partitions
W = 128
WP = W + 4       # padded w stride, data cols [2, 130)
OWN = 64         # owned D slices per core
HALO = 3
DEXT = OWN + 2 * HALO          # 70 slab D-slots
B = 4
EPS = 1e-05

CH = 4                         # D-slots per matmul chunk (512 free)
ACH = 10                       # D-slots per phase-A chunk
N_A = 7                        # phase-A chunks
OLO, OHI = 3, 67               # owned slots
N_S1 = 17                      # stage1 chunks: E slots [1, 69)
N_J = 16                       # owned chunks


def _band(offsets, rep_edges=False):
    m = np.zeros((P, P), np.float32)
    for o in offsets:
        for i in range(P):
            j = i + o
            if 0 <= j < P:
                m[j, i] += 1.0
            elif rep_edges:
                m[min(max(j, 0), P - 1), i] += 1.0
    return m


def _const_mats():
    a1 = _band([-1, 1], rep_edges=True)   # H-neighbor sum, edges replicated
    m_b = a1 - 6.0 * np.eye(P, dtype=np.float32)
    ident = np.eye(P, dtype=np.float32)
    t3 = _band([-1, 0, 1])
    t5 = _band([-2, -1, 0, 1, 2])
    zero = np.zeros((P, P), np.float32)

    def pair(a, b):
        return np.concatenate([a[:, None, :], b[:, None, :]], 1).reshape(P, 2 * P)

    return {
        "s1a": pair(m_b, ident), "s1b": pair(ident, ident),     # fp8e4
        "s1c": pair(ident, zero),
        "s2a": pair(t5, t3), "s2b": pair(t3, t3),               # fp8e5
        "s2c": pair(t3, ident), "s2d": pair(ident, ident),
        "s2e": pair(ident, zero),
        "ones": np.ones((P, P), np.float32),                    # bf16
    }


def _pair_ap(view3, s0, c0, delta):
    """Moving AP for a DoubleRow pair over a [p, s, w]-viewed tile:
    dims [(part), (delta, 2), (WP, CH), (1, W)]; second half of the
    pair reads the same window shifted by `delta` elements."""
    a = view3[:, s0:s0 + CH, c0:c0 + W].copy()
    a.ap.insert(1, [delta, 2])
    return a


# stage2 term list: 13 ball-radius-2 (dz, dw) shifts paired into 7
# DoubleRow matmuls; H-axis taps live in the stationary matrices.
S2 = [("s2a", (0, 0), (-1, 0)),
      ("s2b", (0, -1), (0, 1)),
      ("s2b", (1, 0), (-1, -1)),
      ("s2b", (-1, 1), (1, -1)),
      ("s2c", (1, 1), (0, -2)),
      ("s2d", (0, 2), (-2, 0)),
      ("s2e", (2, 0), (2, 0))]


def _build_program():
    nc = bacc.Bacc("TRN2", target_bir_lowering=False, debug=False,
                   num_devices=8)
    d_out0 = nc.dram_tensor("out0", [P, DEXT * W], fp16, kind="ExternalInput")
    d_out1 = nc.dram_tensor("out1", [P, DEXT * W], fp16, kind="ExternalInput")
    d_t8 = nc.dram_tensor("t8", [P, DEXT * W], fp8e4, kind="ExternalInput")
    d_tw = nc.dram_tensor("tw", [P, OWN * W], fp16, kind="ExternalInput")
    d_m4 = nc.dram_tensor("m4pack", [P, 3 * 2 * P], fp8e4,
                          kind="ExternalInput")
    d_m5 = nc.dram_tensor("m5pack", [P, 5 * 2 * P], fp8e5,
                          kind="ExternalInput")
    d_ones = nc.dram_tensor("ones", [P, P], bf16, kind="ExternalInput")
    d_psums = nc.dram_tensor("psums", [P, 3], f32, kind="ExternalOutput")

    with tile.TileContext(nc) as tc:
        with tc.tile_pool(name="consts", bufs=1) as cp, \
             tc.tile_pool(name="slabs", bufs=1) as sp, \
             tc.tile_pool(name="rings", bufs=3) as rp, \
             tc.tile_pool(name="ps_e", bufs=3, space="PSUM") as ps_e, \
             tc.tile_pool(name="ps_r", bufs=3, space="PSUM") as ps_r, \
             tc.tile_pool(name="ps_a", bufs=1, space="PSUM") as ps_a:

            # packed stationaries: 3 triggers on the idle Pool queue so
            # the Sync queue starts input loads immediately
            m4 = cp.tile([P, 3 * 2 * P], fp8e4, tag="m4", name="m4")
            m5 = cp.tile([P, 5 * 2 * P], fp8e5, tag="m5", name="m5")
            ones = cp.tile([P, P], bf16, tag="ones", name="ones")
            nc.gpsimd.dma_start(m4[:], d_m4[:])
            nc.gpsimd.dma_start(m5[:], d_m5[:])
            nc.gpsimd.dma_start(ones[:], d_ones[:])
            mats = {}
            for i, n in enumerate(("s1a", "s1b", "s1c")):
                mats[n] = m4[:, 2 * P * i:2 * P * (i + 1)].rearrange(
                    "p (two m) -> p two m", two=2)
            for i, n in enumerate(("s2a", "s2b", "s2c", "s2d", "s2e")):
                mats[n] = m5[:, 2 * P * i:2 * P * (i + 1)].rearrange(
                    "p (two m) -> p two m", two=2)

            # --- persistent slabs ---
            out0 = sp.tile([P, DEXT * W], fp16, tag="out0", name="out0")
            out1 = sp.tile([P, DEXT * W], fp16, tag="out1", name="out1")
            t8_t = sp.tile([P, DEXT * W], fp8e4, tag="t8", name="t8")
            t8v = t8_t[:].rearrange("p (s w) -> p s w", w=W)
            v_t = sp.tile([P, DEXT * WP], fp8e4, tag="v", name="v")
            v3 = v_t[:].rearrange("p (s w) -> p s w", w=WP)
            e_t = sp.tile([P, DEXT * WP], fp8e5, tag="e", name="e")
            e3 = e_t[:].rearrange("p (s w) -> p s w", w=WP)
            probs_t = sp.tile([P, OWN * W], bf16, tag="probs", name="probs")
            probs3 = probs_t[:].rearrange("p (s w) -> p s w", w=W)
            tw_t = sp.tile([P, OWN * W], fp16, tag="tw", name="tw")
            acc = sp.tile([P, 3], f32, tag="acc", name="acc")
            acc16 = sp.tile([P, 2 * N_J], f32, tag="acc16", name="acc16")

            out0v = out0[:].rearrange("p (s w) -> p s w", w=W)
            out1v = out1[:].rearrange("p (s w) -> p s w", w=W)

            # --- input DMA, front-loaded and spread across three trigger
            # queues so the first chunk's transfers run in parallel
            for k in range(N_A):
                s0 = k * ACH
                ns = min(ACH, DEXT - s0)
                sl = slice(s0 * W, (s0 + ns) * W)
                nc.sync.dma_start(out0[:, sl], d_out0[:, sl])
                nc.scalar.dma_start(out1[:, sl], d_out1[:, sl])
                nc.gpsimd.dma_start(t8_t[:, sl], d_t8[:, sl])
            for q in range(4):
                sl = slice(q * 16 * W, (q + 1) * 16 * W)
                nc.sync.dma_start(tw_t[:, sl], d_tw[:, sl])

            # E w-pad zeroing (cols 0,1,130,131 never written by stage1)
            for c in (0, 1, 130, 131):
                nc.vector.memset(e3[:, :, c:c + 1], 0.0)

            # --- fused software-pipelined main loop ---
            a_done = [0]

            def phase_a(k):
                s0 = k * ACH
                ns = min(ACH, DEXT - s0)
                diff = rp.tile([P, ACH * W], fp16, tag="diff")
                d3 = diff[:].rearrange("p (s w) -> p s w", w=W)
                nc.vector.tensor_sub(d3[:, :ns, :], out1v[:, s0:s0 + ns, :],
                                     out0v[:, s0:s0 + ns, :])
                # v = (diff > 0) + 8t in {0,1,8,9} (e4m3-exact, carry-free)
                nc.vector.scalar_tensor_tensor(
                    v3[:, s0:s0 + ns, 2:130], d3[:, :ns, :], 0.0,
                    t8v[:, s0:s0 + ns, :], op0=Alu.is_gt, op1=Alu.add)
                # replicated w-pad cols of v (Pool)
                nc.gpsimd.tensor_copy(v3[:, s0:s0 + ns, 1:2],
                                      v3[:, s0:s0 + ns, 2:3])
                nc.gpsimd.tensor_copy(v3[:, s0:s0 + ns, 130:131],
                                      v3[:, s0:s0 + ns, 129:130])
                o0, o1 = max(s0, OLO), min(s0 + ns, OHI)
                if o0 < o1:
                    nc.scalar.activation(probs3[:, o0 - OLO:o1 - OLO, :],
                                         d3[:, o0 - s0:o1 - s0, :],
                                         Act.Sigmoid)

            def need_a(upto_slot):
                while a_done[0] * ACH < upto_slot and a_done[0] < N_A:
                    phase_a(a_done[0])
                    a_done[0] += 1

            acc_ps = ps_a.tile([P, 512], f32, tag="accps", name="accps")
            pending = []   # fld tiles whose PE reduction is delayed a step

            def reduce_flush():
                jf, fldf = pending.pop(0)
                nc.tensor.matmul(acc_ps[:], ones[:], fldf[:],
                                 start=(jf == 0), stop=(jf == N_J - 1),
                                 skip_group_check=True)

            for step in range(N_S1 + 2):
                # PE first sees the delayed reduce (operands a step old),
                # then the uninterrupted fp8 DR stream of this step.
                if pending:
                    reduce_flush()

                g = step
                if g < N_S1:
                    # ---- stage1 chunk: E slots [1+4g, 5+4g) ----
                    s0 = 1 + CH * g
                    need_a(s0 + CH + 1)
                    pe = ps_e.tile([P, CH * W], f32, tag="eps")
                    pe3 = pe[:].rearrange("p (s w) -> p s w", w=W)
                    # 5 cross-terms on v as 3 DR pairs:
                    # (m_b@v(0,0), I@v(0,-1)), (I@v(0,+1), I@v(-1,0)),
                    # (I@v(+1,0), zero)
                    terms = (("s1a", (0, 0), (0, -1)),
                             ("s1b", (0, 1), (-1, 0)),
                             ("s1c", (1, 0), (1, 0)))
                    for i, (st, d1, d2) in enumerate(terms):
                        delta = (d2[0] - d1[0]) * WP + (d2[1] - d1[1])
                        mv = _pair_ap(v3, s0 + d1[0], 2 + d1[1], delta)
                        nc.tensor.matmul(pe3[:], mats[st], mv,
                                         start=(i == 0),
                                         stop=(i == len(terms) - 1),
                                         perf_mode=DR)
                    nc.scalar.activation(e3[:, s0:s0 + CH, 2:130], pe3[:],
                                         Act.Square, scale=1.0 / 8.0)

                j = step - 2
                if 0 <= j < N_J:
                    # ---- stage2 + products: owned slots [OLO+4j, +4) ----
                    s0 = OLO + CH * j
                    pr = ps_r.tile([P, CH * W], f32, tag="rps")
                    pr3 = pr[:].rearrange("p (s w) -> p s w", w=W)
                    for i, (st, d1, d2) in enumerate(S2):
                        delta = (d2[0] - d1[0]) * WP + (d2[1] - d1[1])
                        mv = _pair_ap(e3, s0 + d1[0], 2 + d1[1], delta)
                        nc.tensor.matmul(pr3[:], mats[st], mv,
                                         start=(i == 0),
                                         stop=(i == len(S2) - 1),
                                         perf_mode=DR)
                    m_t = rp.tile([P, 512], bf16, tag="m")
                    nc.scalar.activation(m_t[:], pr[:], Act.Sign)
                    fld = rp.tile([P, 512], bf16, tag="fld")
                    scr = rp.tile([P, 512], bf16, tag="scr")
                    jj = slice(512 * j, 512 * (j + 1))
                    # pm field -> PE ones-reduce (delayed); mt and ptm
                    # reduce per-chunk on DVE via STT accum_out
                    nc.vector.tensor_mul(fld[:], m_t[:], probs_t[:, jj])
                    nc.vector.scalar_tensor_tensor(
                        scr[:], m_t[:], 1.0, tw_t[:, jj],
                        op0=Alu.mult, op1=Alu.mult,
                        accum_out=acc16[:, j:j + 1])
                    nc.vector.scalar_tensor_tensor(
                        scr[:], fld[:], 1.0, tw_t[:, jj],
                        op0=Alu.mult, op1=Alu.mult,
                        accum_out=acc16[:, N_J + j:N_J + j + 1])
                    pending.append((j, fld))
            while pending:
                reduce_flush()

            # ---- final: fold the accumulators to acc[:, 0:3] ----
            scr = rp.tile([P, 512], bf16, tag="scr")
            nc.vector.tensor_scalar(
                scr[:], acc_ps[:], 0.0, None,
                op0=Alu.add, op1=Alu.add, accum_out=acc[:, 0:1])
            scr16 = rp.tile([P, N_J], f32, tag="scr16")
            nc.vector.tensor_scalar(
                scr16[:], acc16[:, 0:N_J], 0.0, None,
                op0=Alu.add, op1=Alu.add, accum_out=acc[:, 1:2])
            nc.vector.tensor_scalar(
                scr16[:], acc16[:, N_J:2 * N_J], 0.0, None,
                op0=Alu.add, op1=Alu.add, accum_out=acc[:, 2:3])
            nc.sync.dma_start(d_psums[:], acc[:])

    nc.compile()
    return nc


_CACHE = {}
TRACE = False
_LAST = {"exec_time_ns": None, "results": None}


def _get_program():
    if "nc" not in _CACHE:
        _CACHE["nc"] = _build_program()
    return _CACHE["nc"]


def last_exec_time_ns():
    return _LAST["exec_time_ns"]


def _core_slabs(output, target, c):
    s, h = c // 2, c % 2
    d0 = 0 if h == 0 else OWN
    sl = slice(d0, d0 + DEXT)
    out_p = np.pad(output[s], ((0, 0), (HALO, HALO), (0, 0), (0, 0)),
                   mode="edge")[:, sl]
    tgt_p = np.pad(target[s, 0], ((HALO, HALO), (0, 0), (0, 0)),
                   mode="edge")[sl]

    def tr(a, dt):  # [D,H,W] -> [H, D*W]
        return np.ascontiguousarray(
            a.transpose(1, 0, 2).astype(dt)).reshape(P, -1)

    tw = tgt_p[HALO:HALO + OWN]
    return {"out0": tr(out_p[0], np.float16),
            "out1": tr(out_p[1], np.float16),
            "t8": tr(8.0 * tgt_p, ml_dtypes.float8_e4m3),
            "tw": tr(tw, np.float16)}


def kernel(output, target):
    output = np.asarray(output, dtype=np.float32)
    target = np.asarray(target, dtype=np.float32)
    nc = _get_program()

    cm = _const_mats()
    mats = {
        "m4pack": np.concatenate(
            [cm[n] for n in ("s1a", "s1b", "s1c")],
            axis=1).astype(ml_dtypes.float8_e4m3),
        "m5pack": np.concatenate(
            [cm[n] for n in ("s2a", "s2b", "s2c", "s2d", "s2e")],
            axis=1).astype(ml_dtypes.float8_e5m2),
        "ones": cm["ones"].astype(ml_dtypes.bfloat16),
    }

    in_maps = []
    for c in range(8):
        im = _core_slabs(output, target, c)
        im.update(mats)
        in_maps.append(im)

    res = run_bass_kernel_spmd(nc, in_maps, list(range(8)), trace=TRACE)
    _LAST["exec_time_ns"] = res.exec_time_ns
    _LAST["results"] = res
    parts = np.zeros((B, 3), np.float64)
    for c in range(8):
        ps = res.results[c]["psums"].astype(np.float64)
        # col 0 (pm) was partition-contracted by the PE ones-reduce
        # (all partitions identical); cols 1,2 are per-partition sums.
        parts[c // 2] += [ps[0, 0], ps[:, 1].sum(), ps[:, 2].sum()]
    s_pm, s_tm, s_ptm = parts.T
    dice = (2.0 * s_ptm + EPS) / (s_pm + s_tm + EPS)
    per_sample = 1.0 - dice
    return np.float32(per_sample.sum() / B)


# revision 40
# speedup vs baseline: 1.0718x; 1.0718x over previous
"""Trainium2 Bass kernel for nn_BoundaryDiceLoss_82171314307268.

Sharding: pure data-parallel over 8 cores; core c handles sample c//2,
D-half c%2. Each core gets a [H=128(partitions), 70 D-slots, 128 w] slab
(64 owned D slices + 3 halo, out-of-volume D replicated with edge
values) of out0/out1 (fp16 logits) plus target fields.

Per-core algorithm (fused chunk pipeline):
  diff = out1 - out0 (fp16, DVE);  probs = sigmoid(diff) (ACT, bf16)
  v = (diff > 0) + 8*target in {0,1,8,9} fp8e4 (DVE stt; all e4m3-exact).
  Boundary: L = cross-sum(v) - 6v;  |L_p-part| <= 6 < 8 <= |8*L_t-part|
    when t-boundary, so L != 0 iff either mask is non-constant on its
    in-volume 6-neighborhood (edge-replicated padding: host D-halo,
    per-chunk w-pad copies, A1 edge rows).  The 5 cross-terms run as 3
    fp8e4 DoubleRow matmul pairs (the pair is an extra AP dim over the
    same v tile at two shift offsets; PE is output-column-bound at
    2.4 GHz so pairing halves instruction count).
    E = Square(L/8) -> fp8e5, > 0 exactly on the two-sided boundary.
  region r = conv3d(E, ball radius 2): 13 (dz,dw) shift-terms as 7
    fp8e5 DoubleRow matmuls accumulating one PSUM chunk; r > 0 is
    exactly the reference's dilation mask (all terms >= 0).
  Products: m = Sign(r) in {0,1} (ACT, exact since r >= 0), pm = m*probs,
    mt = m*t, ptm = pm*t (DVE bf16 TTs); reduced on PE by an all-ones
    bf16 matmul per field accumulating across all 16 owned chunks in
    PSUM (delayed one step so PE never waits on fresh DVE output);
    final 3 DVE accum_out -> [128,3] (all partitions identical).
  The reference's nonempty gate is algebraically redundant (empty
  region => sums exactly 0 => dice = eps/eps = 1 => loss term 0), so
  S_m is never computed.

Engine notes: fp8 DoubleRow = 0.5 PE cycles/row (two matmul terms per
instruction); DVE perf modes need all-2-byte packed SBUF operands
(fp8/f32/PSUM operands drop to 1x); PE runs 2.4 GHz only when kept
continuously busy, hence one fused software-pipelined loop rather than
phase barriers. Input DMAs + tiny pad copies issue from the idle Pool
queue (25ns dispatch vs 625ns on sync).
"""
import sys

sys.path.insert(0, "/opt/trn_rl_repo")

import numpy as np
import ml_dtypes

import concourse.bass as bass
import concourse.bacc as bacc
import concourse.tile as tile
import concourse.mybir as mybir
from concourse.bass_utils import run_bass_kernel_spmd

f32 = mybir.dt.float32
bf16 = mybir.dt.bfloat16
fp16 = mybir.dt.float16
fp8e4 = mybir.dt.float8e4
fp8e5 = mybir.dt.float8e5
Alu = mybir.AluOpType
Act = mybir.ActivationFunctionType
DR = mybir.MatmulPerfMode.DoubleRow

P = 128          # H on partitions
W = 128
WP = W + 4       # padded w stride, data cols [2, 130)
OWN = 64         # owned D slices per core
HALO = 3
DEXT = OWN + 2 * HALO          # 70 slab D-slots
B = 4
EPS = 1e-05

CH = 4                         # D-slots per matmul chunk (512 free)
# phase-A chunk boundaries: small first chunk so the pipeline head
# (stage1 of chunk 0 needs slots [0,6)) starts as early as possible
A_BOUNDS = [0, 6, 14, 22, 30, 38, 46, 54, 62, 70]
N_A = len(A_BOUNDS) - 1
ACH_MAX = 8
OLO, OHI = 3, 67               # owned slots
N_S1 = 17                      # stage1 chunks: E slots [1, 69)
N_J = 16                       # owned chunks


def _band(offsets, rep_edges=False):
    m = np.zeros((P, P), np.float32)
    for o in offsets:
        for i in range(P):
            j = i + o
            if 0 <= j < P:
                m[j, i] += 1.0
            elif rep_edges:
                m[min(max(j, 0), P - 1), i] += 1.0
    return m


def _const_mats():
    a1 = _band([-1, 1], rep_edges=True)   # H-neighbor sum, edges replicated
    m_b = a1 - 6.0 * np.eye(P, dtype=np.float32)
    ident = np.eye(P, dtype=np.float32)
    t3 = _band([-1, 0, 1])
    t5 = _band([-2, -1, 0, 1, 2])
    zero = np.zeros((P, P), np.float32)

    def pair(a, b):
        return np.concatenate([a[:, None, :], b[:, None, :]], 1).reshape(P, 2 * P)

    return {
        "s1a": pair(m_b, ident), "s1b": pair(ident, ident),     # fp8e4
        "s1c": pair(ident, zero),
        "s2a": pair(t5, t3), "s2b": pair(t3, t3),               # fp8e5
        "s2c": pair(t3, ident), "s2d": pair(ident, ident),
        "s2e": pair(ident, zero),
        "ones": np.ones((P, P), np.float32),                    # bf16
    }


def _pair_ap(view3, s0, c0, delta):
    """Moving AP for a DoubleRow pair over a [p, s, w]-viewed tile:
    dims [(part), (delta, 2), (WP, CH), (1, W)]; second half of the
    pair reads the same window shifted by `delta` elements."""
    a = view3[:, s0:s0 + CH, c0:c0 + W].copy()
    a.ap.insert(1, [delta, 2])
    return a


# stage2 term list: 13 ball-radius-2 (dz, dw) shifts paired into 7
# DoubleRow matmuls; H-axis taps live in the stationary matrices.
S2 = [("s2a", (0, 0), (-1, 0)),
      ("s2b", (0, -1), (0, 1)),
      ("s2b", (1, 0), (-1, -1)),
      ("s2b", (-1, 1), (1, -1)),
      ("s2c", (1, 1), (0, -2)),
      ("s2d", (0, 2), (-2, 0)),
      ("s2e", (2, 0), (2, 0))]


def _build_program():
    nc = bacc.Bacc("TRN2", target_bir_lowering=False, debug=False,
                   num_devices=8)
    d_out0 = nc.dram_tensor("out0", [P, DEXT * W], fp16, kind="ExternalInput")
    d_out1 = nc.dram_tensor("out1", [P, DEXT * W], fp16, kind="ExternalInput")
    d_t8 = nc.dram_tensor("t8", [P, DEXT * W], fp8e4, kind="ExternalInput")
    d_tw = nc.dram_tensor("tw", [P, OWN * W], fp16, kind="ExternalInput")
    d_m4 = nc.dram_tensor("m4pack", [P, 3 * 2 * P], fp8e4,
                          kind="ExternalInput")
    d_m5 = nc.dram_tensor("m5pack", [P, 5 * 2 * P], fp8e5,
                          kind="ExternalInput")
    d_ones = nc.dram_tensor("ones", [P, P], bf16, kind="ExternalInput")
    d_psums = nc.dram_tensor("psums", [P, 3], f32, kind="ExternalOutput")

    with tile.TileContext(nc) as tc:
        with tc.tile_pool(name="consts", bufs=1) as cp, \
             tc.tile_pool(name="slabs", bufs=1) as sp, \
             tc.tile_pool(name="rings", bufs=3) as rp, \
             tc.tile_pool(name="ps_e", bufs=2, space="PSUM") as ps_e, \
             tc.tile_pool(name="ps_r", bufs=3, space="PSUM") as ps_r, \
             tc.tile_pool(name="ps_a", bufs=1, space="PSUM") as ps_a:

            # packed stationaries on the Pool queue so the Sync queue
            # starts input loads immediately (triggers emitted below,
            # after the first t8 chunk's)
            m4 = cp.tile([P, 3 * 2 * P], fp8e4, tag="m4", name="m4")
            m5 = cp.tile([P, 5 * 2 * P], fp8e5, tag="m5", name="m5")
            ones = cp.tile([P, P], bf16, tag="ones", name="ones")
            mats = {}
            for i, n in enumerate(("s1a", "s1b", "s1c")):
                mats[n] = m4[:, 2 * P * i:2 * P * (i + 1)].rearrange(
                    "p (two m) -> p two m", two=2)
            for i, n in enumerate(("s2a", "s2b", "s2c", "s2d", "s2e")):
                mats[n] = m5[:, 2 * P * i:2 * P * (i + 1)].rearrange(
                    "p (two m) -> p two m", two=2)

            # --- persistent slabs ---
            out0 = sp.tile([P, DEXT * W], fp16, tag="out0", name="out0")
            out1 = sp.tile([P, DEXT * W], fp16, tag="out1", name="out1")
            t8_t = sp.tile([P, DEXT * W], fp8e4, tag="t8", name="t8")
            t8v = t8_t[:].rearrange("p (s w) -> p s w", w=W)
            v_t = sp.tile([P, DEXT * WP], fp8e4, tag="v", name="v")
            v3 = v_t[:].rearrange("p (s w) -> p s w", w=WP)
            e_t = sp.tile([P, DEXT * WP], fp8e5, tag="e", name="e")
            e3 = e_t[:].rearrange("p (s w) -> p s w", w=WP)
            probs_t = sp.tile([P, OWN * W], bf16, tag="probs", name="probs")
            probs3 = probs_t[:].rearrange("p (s w) -> p s w", w=W)
            tw_t = sp.tile([P, OWN * W], fp16, tag="tw", name="tw")
            acc = sp.tile([P, 3], f32, tag="acc", name="acc")

            out0v = out0[:].rearrange("p (s w) -> p s w", w=W)
            out1v = out1[:].rearrange("p (s w) -> p s w", w=W)

            # --- input DMA, front-loaded and spread across three trigger
            # queues so the first chunk's transfers run in parallel
            for k in range(N_A):
                s0, s1 = A_BOUNDS[k], A_BOUNDS[k + 1]
                sl = slice(s0 * W, s1 * W)
                nc.sync.dma_start(out0[:, sl], d_out0[:, sl])
                nc.scalar.dma_start(out1[:, sl], d_out1[:, sl])
                nc.gpsimd.dma_start(t8_t[:, sl], d_t8[:, sl])
                if k == 0:
                    nc.gpsimd.dma_start(m4[:], d_m4[:])
                    nc.gpsimd.dma_start(m5[:], d_m5[:])
                    nc.gpsimd.dma_start(ones[:], d_ones[:])
            for q in range(4):
                sl = slice(q * 16 * W, (q + 1) * 16 * W)
                nc.sync.dma_start(tw_t[:, sl], d_tw[:, sl])

            # E w-pad zeroing (cols 0,1,130,131 never written by stage1)
            for c in (0, 1, 130, 131):
                nc.vector.memset(e3[:, :, c:c + 1], 0.0)

            # --- fused software-pipelined main loop ---
            a_done = [0]

            def phase_a(k):
                s0 = A_BOUNDS[k]
                ns = A_BOUNDS[k + 1] - s0
                diff = rp.tile([P, ACH_MAX * W], fp16, tag="diff")
                d3 = diff[:].rearrange("p (s w) -> p s w", w=W)
                nc.vector.tensor_sub(d3[:, :ns, :], out1v[:, s0:s0 + ns, :],
                                     out0v[:, s0:s0 + ns, :])
                # v = (diff > 0) + 8t in {0,1,8,9} (e4m3-exact, carry-free)
                nc.vector.scalar_tensor_tensor(
                    v3[:, s0:s0 + ns, 2:130], d3[:, :ns, :], 0.0,
                    t8v[:, s0:s0 + ns, :], op0=Alu.is_gt, op1=Alu.add)
                # replicated w-pad cols of v (Pool)
                nc.gpsimd.tensor_copy(v3[:, s0:s0 + ns, 1:2],
                                      v3[:, s0:s0 + ns, 2:3])
                nc.gpsimd.tensor_copy(v3[:, s0:s0 + ns, 130:131],
                                      v3[:, s0:s0 + ns, 129:130])
                o0, o1 = max(s0, OLO), min(s0 + ns, OHI)
                if o0 < o1:
                    nc.scalar.activation(probs3[:, o0 - OLO:o1 - OLO, :],
                                         d3[:, o0 - s0:o1 - s0, :],
                                         Act.Sigmoid)

            def need_a(upto_slot):
                while a_done[0] < N_A and A_BOUNDS[a_done[0]] < upto_slot:
                    phase_a(a_done[0])
                    a_done[0] += 1

            acc_ps = [ps_a.tile([P, 512], f32, tag=f"acc{i}", name=f"acc{i}")
                      for i in range(3)]
            pending = []   # fld tiles whose PE reduction is delayed a step

            def reduce_flush():
                jf, fldf = pending.pop(0)
                for i in range(3):
                    nc.tensor.matmul(acc_ps[i][:], ones[:],
                                     fldf[:, 512 * i:512 * (i + 1)],
                                     start=(jf == 0), stop=(jf == N_J - 1),
                                     skip_group_check=True)

            for step in range(N_S1 + 2):
                # PE first sees the delayed reduce (operands a step old),
                # then the uninterrupted fp8 DR stream of this step.
                if pending:
                    reduce_flush()

                g = step
                if g < N_S1:
                    # ---- stage1 chunk: E slots [1+4g, 5+4g) ----
                    s0 = 1 + CH * g
                    need_a(s0 + CH + 1)
                    pe = ps_e.tile([P, CH * W], f32, tag="eps")
                    pe3 = pe[:].rearrange("p (s w) -> p s w", w=W)
                    # 5 cross-terms on v as 3 DR pairs:
                    # (m_b@v(0,0), I@v(0,-1)), (I@v(0,+1), I@v(-1,0)),
                    # (I@v(+1,0), zero)
                    terms = (("s1a", (0, 0), (0, -1)),
                             ("s1b", (0, 1), (-1, 0)),
                             ("s1c", (1, 0), (1, 0)))
                    for i, (st, d1, d2) in enumerate(terms):
                        delta = (d2[0] - d1[0]) * WP + (d2[1] - d1[1])
                        mv = _pair_ap(v3, s0 + d1[0], 2 + d1[1], delta)
                        nc.tensor.matmul(pe3[:], mats[st], mv,
                                         start=(i == 0),
                                         stop=(i == len(terms) - 1),
                                         perf_mode=DR)
                    nc.scalar.activation(e3[:, s0:s0 + CH, 2:130], pe3[:],
                                         Act.Square, scale=1.0 / 8.0)

                j = step - 2
                if 0 <= j < N_J:
                    # ---- stage2 + products: owned slots [OLO+4j, +4) ----
                    s0 = OLO + CH * j
                    pr = ps_r.tile([P, CH * W], f32, tag="rps")
                    pr3 = pr[:].rearrange("p (s w) -> p s w", w=W)
                    for i, (st, d1, d2) in enumerate(S2):
                        delta = (d2[0] - d1[0]) * WP + (d2[1] - d1[1])
                        mv = _pair_ap(e3, s0 + d1[0], 2 + d1[1], delta)
                        nc.tensor.matmul(pr3[:], mats[st], mv,
                                         start=(i == 0),
                                         stop=(i == len(S2) - 1),
                                         perf_mode=DR)
                    m_t = rp.tile([P, 512], bf16, tag="m")
                    nc.scalar.activation(m_t[:], pr[:], Act.Sign)
                    fld = rp.tile([P, 3 * 512], bf16, tag="fld")
                    jj = slice(512 * j, 512 * (j + 1))
                    nc.vector.tensor_mul(fld[:, 0:512], m_t[:],
                                         probs_t[:, jj])
                    nc.vector.tensor_mul(fld[:, 512:1024], m_t[:],
                                         tw_t[:, jj])
                    nc.vector.tensor_mul(fld[:, 1024:1536], fld[:, 0:512],
                                         tw_t[:, jj])
                    pending.append((j, fld))
            while pending:
                reduce_flush()

            # ---- final: fold the PE accumulators to acc[:, 0:3] ----
            scr = rp.tile([P, 512], bf16, tag="scr")
            for k in range(3):
                nc.vector.tensor_scalar(
                    scr[:], acc_ps[k][:], 0.0, None,
                    op0=Alu.add, op1=Alu.add, accum_out=acc[:, k:k + 1])
            nc.sync.dma_start(d_psums[:], acc[:])

    nc.compile()
    return nc


_CACHE = {}
TRACE = False
_LAST = {"exec_time_ns": None, "results": None}


def _get_program():
    if "nc" not in _CACHE:
        _CACHE["nc"] = _build_program()
    return _CACHE["nc"]


def last_exec_time_ns():
    return _LAST["exec_time_ns"]


def _core_slabs(output, target, c):
    s, h = c // 2, c % 2
    d0 = 0 if h == 0 else OWN
    sl = slice(d0, d0 + DEXT)
    out_p = np.pad(output[s], ((0, 0), (HALO, HALO), (0, 0), (0, 0)),
                   mode="edge")[:, sl]
    tgt_p = np.pad(target[s, 0], ((HALO, HALO), (0, 0), (0, 0)),
                   mode="edge")[sl]

    def tr(a, dt):  # [D,H,W] -> [H, D*W]
        return np.ascontiguousarray(
            a.transpose(1, 0, 2).astype(dt)).reshape(P, -1)

    tw = tgt_p[HALO:HALO + OWN]
    return {"out0": tr(out_p[0], np.float16),
            "out1": tr(out_p[1], np.float16),
            "t8": tr(8.0 * tgt_p, ml_dtypes.float8_e4m3),
            "tw": tr(tw, np.float16)}


def kernel(output, target):
    output = np.asarray(output, dtype=np.float32)
    target = np.asarray(target, dtype=np.float32)
    nc = _get_program()

    cm = _const_mats()
    mats = {
        "m4pack": np.concatenate(
            [cm[n] for n in ("s1a", "s1b", "s1c")],
            axis=1).astype(ml_dtypes.float8_e4m3),
        "m5pack": np.concatenate(
            [cm[n] for n in ("s2a", "s2b", "s2c", "s2d", "s2e")],
            axis=1).astype(ml_dtypes.float8_e5m2),
        "ones": cm["ones"].astype(ml_dtypes.bfloat16),
    }

    in_maps = []
    for c in range(8):
        im = _core_slabs(output, target, c)
        im.update(mats)
        in_maps.append(im)

    res = run_bass_kernel_spmd(nc, in_maps, list(range(8)), trace=TRACE)
    _LAST["exec_time_ns"] = res.exec_time_ns
    _LAST["results"] = res
    parts = np.zeros((B, 3), np.float64)
    for c in range(8):
        parts[c // 2] += res.results[c]["psums"][0].astype(np.float64)
    s_pm, s_tm, s_ptm = parts.T
    dice = (2.0 * s_ptm + EPS) / (s_pm + s_tm + EPS)
    per_sample = 1.0 - dice
    return np.float32(per_sample.sum() / B)


# revision 42
# speedup vs baseline: 1.1976x; 1.1174x over previous
"""Trainium2 Bass kernel for nn_BoundaryDiceLoss_82171314307268.

Sharding: pure data-parallel over 8 cores; core c handles sample c//2,
D-half c%2. Each core gets a [H=128(partitions), 70 D-slots, 128 w] slab
(64 owned D slices + 3 halo, out-of-volume D replicated with edge
values) of out0/out1 (fp16 logits) plus target fields.

Per-core algorithm (fused chunk pipeline):
  diff = out1 - out0 (fp16, DVE);  probs = sigmoid(diff) (ACT, bf16)
  v = (diff > 0) + 8*target in {0,1,8,9} fp8e4 (DVE stt; all e4m3-exact).
  Boundary: L = cross-sum(v) - 6v;  |L_p-part| <= 6 < 8 <= |8*L_t-part|
    when t-boundary, so L != 0 iff either mask is non-constant on its
    in-volume 6-neighborhood (edge-replicated padding: host D-halo,
    per-chunk w-pad copies, A1 edge rows).  The 5 cross-terms run as 3
    fp8e4 DoubleRow matmul pairs (the pair is an extra AP dim over the
    same v tile at two shift offsets; PE is output-column-bound at
    2.4 GHz so pairing halves instruction count).
    E = Square(L/8) -> fp8e5, > 0 exactly on the two-sided boundary.
  region r = conv3d(E, ball radius 2): 13 (dz,dw) shift-terms as 7
    fp8e5 DoubleRow matmuls accumulating one PSUM chunk; r > 0 is
    exactly the reference's dilation mask (all terms >= 0).
  Products: m = Sign(r) in {0,1} (ACT, exact since r >= 0), pm = m*probs,
    mt = m*t, ptm = pm*t (DVE bf16 TTs); reduced on PE by an all-ones
    bf16 matmul per field accumulating across all 16 owned chunks in
    PSUM (delayed one step so PE never waits on fresh DVE output);
    final 3 DVE accum_out -> [128,3] (all partitions identical).
  The reference's nonempty gate is algebraically redundant (empty
  region => sums exactly 0 => dice = eps/eps = 1 => loss term 0), so
  S_m is never computed.

Engine notes: fp8 DoubleRow = 0.5 PE cycles/row (two matmul terms per
instruction); DVE perf modes need all-2-byte packed SBUF operands
(fp8/f32/PSUM operands drop to 1x); PE runs 2.4 GHz only when kept
continuously busy, hence one fused software-pipelined loop rather than
phase barriers. Input DMAs + tiny pad copies issue from the idle Pool
queue (25ns dispatch vs 625ns on sync).
"""
import sys

sys.path.insert(0, "/opt/trn_rl_repo")

import numpy as np
import ml_dtypes

import concourse.bass as bass
import concourse.bacc as bacc
import concourse.tile as tile
import concourse.mybir as mybir
from concourse.bass_utils import run_bass_kernel_spmd

f32 = mybir.dt.float32
bf16 = mybir.dt.bfloat16
fp16 = mybir.dt.float16
fp8e4 = mybir.dt.float8e4
fp8e5 = mybir.dt.float8e5
Alu = mybir.AluOpType
Act = mybir.ActivationFunctionType
DR = mybir.MatmulPerfMode.DoubleRow

P = 128          # H on partitions
W = 128
WP = W + 4       # padded w stride, data cols [2, 130)
OWN = 64         # owned D slices per core
HALO = 3
DEXT = OWN + 2 * HALO          # 70 slab D-slots
B = 4
EPS = 1e-05

CH = 4                         # D-slots per matmul chunk (512 free)
# phase-A chunk boundaries: small first chunk so the pipeline head
# (stage1 of chunk 0 needs slots [0,6)) starts as early as possible
A_BOUNDS = [0, 6, 16, 26, 36, 46, 56, 66, 70]
N_A = len(A_BOUNDS) - 1
ACH_MAX = 10
OLO, OHI = 3, 67               # owned slots
N_S1 = 17                      # stage1 chunks: E slots [1, 69)
N_J = 16                       # owned chunks


def _band(offsets, rep_edges=False):
    m = np.zeros((P, P), np.float32)
    for o in offsets:
        for i in range(P):
            j = i + o
            if 0 <= j < P:
                m[j, i] += 1.0
            elif rep_edges:
                m[min(max(j, 0), P - 1), i] += 1.0
    return m


def _const_mats():
    a1 = _band([-1, 1], rep_edges=True)   # H-neighbor sum, edges replicated
    m_b = a1 - 6.0 * np.eye(P, dtype=np.float32)
    ident = np.eye(P, dtype=np.float32)
    t3 = _band([-1, 0, 1])
    t5 = _band([-2, -1, 0, 1, 2])
    zero = np.zeros((P, P), np.float32)

    def pair(a, b):
        return np.concatenate([a[:, None, :], b[:, None, :]], 1).reshape(P, 2 * P)

    return {
        "s1a": pair(m_b, ident), "s1b": pair(ident, ident),     # fp8e4
        "s1c": pair(ident, zero),
        "s2a": pair(t5, t3), "s2b": pair(t3, t3),               # fp8e5
        "s2c": pair(t3, ident), "s2d": pair(ident, ident),
        "s2e": pair(ident, zero),
        "ones": np.ones((P, P), np.float32),                    # bf16
    }


def _pair_ap(view3, s0, c0, delta):
    """Moving AP for a DoubleRow pair over a [p, s, w]-viewed tile:
    dims [(part), (delta, 2), (WP, CH), (1, W)]; second half of the
    pair reads the same window shifted by `delta` elements."""
    a = view3[:, s0:s0 + CH, c0:c0 + W].copy()
    a.ap.insert(1, [delta, 2])
    return a


# stage2 term list: 13 ball-radius-2 (dz, dw) shifts paired into 7
# DoubleRow matmuls; H-axis taps live in the stationary matrices.
S2 = [("s2a", (0, 0), (-1, 0)),
      ("s2b", (0, -1), (0, 1)),
      ("s2b", (1, 0), (-1, -1)),
      ("s2b", (-1, 1), (1, -1)),
      ("s2c", (1, 1), (0, -2)),
      ("s2d", (0, 2), (-2, 0)),
      ("s2e", (2, 0), (2, 0))]


def _build_program():
    nc = bacc.Bacc("TRN2", target_bir_lowering=False, debug=False,
                   num_devices=8)
    d_out0 = nc.dram_tensor("out0", [P, DEXT * W], fp16, kind="ExternalInput")
    d_out1 = nc.dram_tensor("out1", [P, DEXT * W], fp16, kind="ExternalInput")
    d_t8 = nc.dram_tensor("t8", [P, DEXT * W], fp8e4, kind="ExternalInput")
    d_tw = nc.dram_tensor("tw", [P, OWN * W], fp16, kind="ExternalInput")
    d_m4 = nc.dram_tensor("m4pack", [P, 3 * 2 * P], fp8e4,
                          kind="ExternalInput")
    d_m5 = nc.dram_tensor("m5pack", [P, 5 * 2 * P], fp8e5,
                          kind="ExternalInput")
    d_ones = nc.dram_tensor("ones", [P, P], bf16, kind="ExternalInput")
    d_psums = nc.dram_tensor("psums", [P, 3], f32, kind="ExternalOutput")

    with tile.TileContext(nc) as tc:
        with tc.tile_pool(name="consts", bufs=1) as cp, \
             tc.tile_pool(name="slabs", bufs=1) as sp, \
             tc.tile_pool(name="rings", bufs=3) as rp, \
             tc.tile_pool(name="ps_e", bufs=2, space="PSUM") as ps_e, \
             tc.tile_pool(name="ps_r", bufs=3, space="PSUM") as ps_r, \
             tc.tile_pool(name="ps_a", bufs=1, space="PSUM") as ps_a:

            # packed stationaries on the Pool queue so the Sync queue
            # starts input loads immediately (triggers emitted below,
            # after the first t8 chunk's)
            m4 = cp.tile([P, 3 * 2 * P], fp8e4, tag="m4", name="m4")
            m5 = cp.tile([P, 5 * 2 * P], fp8e5, tag="m5", name="m5")
            ones = cp.tile([P, P], bf16, tag="ones", name="ones")
            mats = {}
            for i, n in enumerate(("s1a", "s1b", "s1c")):
                mats[n] = m4[:, 2 * P * i:2 * P * (i + 1)].rearrange(
                    "p (two m) -> p two m", two=2)
            for i, n in enumerate(("s2a", "s2b", "s2c", "s2d", "s2e")):
                mats[n] = m5[:, 2 * P * i:2 * P * (i + 1)].rearrange(
                    "p (two m) -> p two m", two=2)

            # --- persistent slabs ---
            out0 = sp.tile([P, DEXT * W], fp16, tag="out0", name="out0")
            out1 = sp.tile([P, DEXT * W], fp16, tag="out1", name="out1")
            t8_t = sp.tile([P, DEXT * W], fp8e4, tag="t8", name="t8")
            t8v = t8_t[:].rearrange("p (s w) -> p s w", w=W)
            v_t = sp.tile([P, DEXT * WP], fp8e4, tag="v", name="v")
            v3 = v_t[:].rearrange("p (s w) -> p s w", w=WP)
            e_t = sp.tile([P, DEXT * WP], fp8e5, tag="e", name="e")
            e3 = e_t[:].rearrange("p (s w) -> p s w", w=WP)
            probs_t = sp.tile([P, OWN * W], bf16, tag="probs", name="probs")
            probs3 = probs_t[:].rearrange("p (s w) -> p s w", w=W)
            tw_t = sp.tile([P, OWN * W], fp16, tag="tw", name="tw")
            acc = sp.tile([P, 3], f32, tag="acc", name="acc")

            out0v = out0[:].rearrange("p (s w) -> p s w", w=W)
            out1v = out1[:].rearrange("p (s w) -> p s w", w=W)

            # --- input DMA, front-loaded and spread across three trigger
            # queues so the first chunk's transfers run in parallel
            for k in range(N_A):
                s0, s1 = A_BOUNDS[k], A_BOUNDS[k + 1]
                sl = slice(s0 * W, s1 * W)
                nc.sync.dma_start(out0[:, sl], d_out0[:, sl])
                nc.sync.dma_start(out1[:, sl], d_out1[:, sl])
                nc.gpsimd.dma_start(t8_t[:, sl], d_t8[:, sl])
                if k == 0:
                    nc.gpsimd.dma_start(m4[:], d_m4[:])
                    nc.gpsimd.dma_start(m5[:], d_m5[:])
                    nc.gpsimd.dma_start(ones[:], d_ones[:])
            for q in range(4):
                sl = slice(q * 16 * W, (q + 1) * 16 * W)
                nc.sync.dma_start(tw_t[:, sl], d_tw[:, sl])

            # E w-pad zeroing (cols 0,1,130,131 never written by stage1)
            for c in (0, 1, 130, 131):
                nc.vector.memset(e3[:, :, c:c + 1], 0.0)

            # --- fused software-pipelined main loop ---
            a_done = [0]

            def phase_a(k):
                s0 = A_BOUNDS[k]
                ns = A_BOUNDS[k + 1] - s0
                diff = rp.tile([P, ACH_MAX * W], fp16, tag="diff")
                d3 = diff[:].rearrange("p (s w) -> p s w", w=W)
                nc.vector.tensor_sub(d3[:, :ns, :], out1v[:, s0:s0 + ns, :],
                                     out0v[:, s0:s0 + ns, :])
                # v = (diff > 0) + 8t in {0,1,8,9} (e4m3-exact, carry-free)
                nc.vector.scalar_tensor_tensor(
                    v3[:, s0:s0 + ns, 2:130], d3[:, :ns, :], 0.0,
                    t8v[:, s0:s0 + ns, :], op0=Alu.is_gt, op1=Alu.add)
                # replicated w-pad cols of v (Pool)
                nc.gpsimd.tensor_copy(v3[:, s0:s0 + ns, 1:2],
                                      v3[:, s0:s0 + ns, 2:3])
                nc.gpsimd.tensor_copy(v3[:, s0:s0 + ns, 130:131],
                                      v3[:, s0:s0 + ns, 129:130])
                o0, o1 = max(s0, OLO), min(s0 + ns, OHI)
                if o0 < o1:
                    nc.scalar.activation(probs3[:, o0 - OLO:o1 - OLO, :],
                                         d3[:, o0 - s0:o1 - s0, :],
                                         Act.Sigmoid)

            def need_a(upto_slot):
                while a_done[0] < N_A and A_BOUNDS[a_done[0]] < upto_slot:
                    phase_a(a_done[0])
                    a_done[0] += 1

            acc_ps = [ps_a.tile([P, 512], f32, tag=f"acc{i}", name=f"acc{i}")
                      for i in range(3)]
            pending = []   # fld tiles whose PE reduction is delayed a step

            def reduce_flush():
                jf, fldf = pending.pop(0)
                for i in range(3):
                    nc.tensor.matmul(acc_ps[i][:], ones[:],
                                     fldf[:, 512 * i:512 * (i + 1)],
                                     start=(jf == 0), stop=(jf == N_J - 1),
                                     skip_group_check=True)

            for step in range(N_S1 + 2):
                # PE first sees the delayed reduce (operands a step old),
                # then the uninterrupted fp8 DR stream of this step.
                if pending:
                    reduce_flush()

                g = step
                if g < N_S1:
                    # ---- stage1 chunk: E slots [1+4g, 5+4g) ----
                    s0 = 1 + CH * g
                    need_a(s0 + CH + 1)
                    pe = ps_e.tile([P, CH * W], f32, tag="eps")
                    pe3 = pe[:].rearrange("p (s w) -> p s w", w=W)
                    # 5 cross-terms on v as 3 DR pairs:
                    # (m_b@v(0,0), I@v(0,-1)), (I@v(0,+1), I@v(-1,0)),
                    # (I@v(+1,0), zero)
                    terms = (("s1a", (0, 0), (0, -1)),
                             ("s1b", (0, 1), (-1, 0)),
                             ("s1c", (1, 0), (1, 0)))
                    for i, (st, d1, d2) in enumerate(terms):
                        delta = (d2[0] - d1[0]) * WP + (d2[1] - d1[1])
                        mv = _pair_ap(v3, s0 + d1[0], 2 + d1[1], delta)
                        nc.tensor.matmul(pe3[:], mats[st], mv,
                                         start=(i == 0),
                                         stop=(i == len(terms) - 1),
                                         perf_mode=DR)
                    nc.scalar.activation(e3[:, s0:s0 + CH, 2:130], pe3[:],
                                         Act.Square, scale=1.0 / 8.0)

                j = step - 2
                if 0 <= j < N_J:
                    # ---- stage2 + products: owned slots [OLO+4j, +4) ----
                    s0 = OLO + CH * j
                    pr = ps_r.tile([P, CH * W], f32, tag="rps")
                    pr3 = pr[:].rearrange("p (s w) -> p s w", w=W)
                    for i, (st, d1, d2) in enumerate(S2):
                        delta = (d2[0] - d1[0]) * WP + (d2[1] - d1[1])
                        mv = _pair_ap(e3, s0 + d1[0], 2 + d1[1], delta)
                        nc.tensor.matmul(pr3[:], mats[st], mv,
                                         start=(i == 0),
                                         stop=(i == len(S2) - 1),
                                         perf_mode=DR)
                    m_t = rp.tile([P, 512], bf16, tag="m")
                    nc.scalar.activation(m_t[:], pr[:], Act.Sign)
                    fld = rp.tile([P, 3 * 512], bf16, tag="fld")
                    jj = slice(512 * j, 512 * (j + 1))
                    nc.vector.tensor_mul(fld[:, 0:512], m_t[:],
                                         probs_t[:, jj])
                    nc.vector.tensor_mul(fld[:, 512:1024], m_t[:],
                                         tw_t[:, jj])
                    nc.vector.tensor_mul(fld[:, 1024:1536], fld[:, 0:512],
                                         tw_t[:, jj])
                    pending.append((j, fld))
            while pending:
                reduce_flush()

            # ---- final: fold the PE accumulators to acc[:, 0:3] ----
            scr = rp.tile([P, 512], bf16, tag="scr")
            for k in range(3):
                nc.vector.tensor_scalar(
                    scr[:], acc_ps[k][:], 0.0, None,
                    op0=Alu.add, op1=Alu.add, accum_out=acc[:, k:k + 1])
            nc.sync.dma_start(d_psums[:], acc[:])

    nc.compile()
    return nc


_CACHE = {}
TRACE = False
_LAST = {"exec_time_ns": None, "results": None}


def _get_program():
    if "nc" not in _CACHE:
        _CACHE["nc"] = _build_program()
    return _CACHE["nc"]


def last_exec_time_ns():
    return _LAST["exec_time_ns"]


def _core_slabs(output, target, c):
    s, h = c // 2, c % 2
    d0 = 0 if h == 0 else OWN
    sl = slice(d0, d0 + DEXT)
    out_p = np.pad(output[s], ((0, 0), (HALO, HALO), (0, 0), (0, 0)),
                   mode="edge")[:, sl]
    tgt_p = np.pad(target[s, 0], ((HALO, HALO), (0, 0), (0, 0)),
                   mode="edge")[sl]

    def tr(a, dt):  # [D,H,W] -> [H, D*W]
        return np.ascontiguousarray(
            a.transpose(1, 0, 2).astype(dt)).reshape(P, -1)

    tw = tgt_p[HALO:HALO + OWN]
    return {"out0": tr(out_p[0], np.float16),
            "out1": tr(out_p[1], np.float16),
            "t8": tr(8.0 * tgt_p, ml_dtypes.float8_e4m3),
            "tw": tr(tw, np.float16)}


def kernel(output, target):
    output = np.asarray(output, dtype=np.float32)
    target = np.asarray(target, dtype=np.float32)
    nc = _get_program()

    cm = _const_mats()
    mats = {
        "m4pack": np.concatenate(
            [cm[n] for n in ("s1a", "s1b", "s1c")],
            axis=1).astype(ml_dtypes.float8_e4m3),
        "m5pack": np.concatenate(
            [cm[n] for n in ("s2a", "s2b", "s2c", "s2d", "s2e")],
            axis=1).astype(ml_dtypes.float8_e5m2),
        "ones": cm["ones"].astype(ml_dtypes.bfloat16),
    }

    in_maps = []
    for c in range(8):
        im = _core_slabs(output, target, c)
        im.update(mats)
        in_maps.append(im)

    res = run_bass_kernel_spmd(nc, in_maps, list(range(8)), trace=TRACE)
    _LAST["exec_time_ns"] = res.exec_time_ns
    _LAST["results"] = res
    parts = np.zeros((B, 3), np.float64)
    for c in range(8):
        parts[c // 2] += res.results[c]["psums"][0].astype(np.float64)
    s_pm, s_tm, s_ptm = parts.T
    dice = (2.0 * s_ptm + EPS) / (s_pm + s_tm + EPS)
    per_sample = 1.0 - dice
    return np.float32(per_sample.sum() / B)
